# revision 51
# baseline (speedup 1.0000x reference)
"""Trainium2 Bass kernel for nn_HODE_MDP (hypergraph ODE message passing).

Math (T_UP = T_GEO = T_P2P = 1.0, ALPHA = 0.8):
    pe  = poi_emb_weight[:-1]                      # [P, D]
    x/s/g = pe * sigmoid(pe @ W_t + b_t)           # col / seq / geo gates
    hg_pois    = x + HG_pu @ (HG_up @ x)
    geo_pois   = g + 0.4 * (poi_geo_graph @ g)
    trans_pois = s + HG_poi_src @ (HG_poi_tar @ s)
    hg_users   = (HG_up @ hg_pois)[user_idx]
    geo_users  = (HG_up @ geo_pois)[user_idx]
    out = concat([hg_pois, geo_pois, trans_pois, hg_users, geo_users])

Distribution (8 NeuronCores), v3 — two collectives only:
  * Full gates are computed on every core (collective latency on this
    part measured ~55us serial, far worse than replicating the work):
    bf16 panel pipeline over 1024-col panels of zT = W.T @ peT, sigmoid
    (scalar), pe*sig (vector), PE-transpose into natural fp8 k-tiles
    with the 2^6 fp8 scale folded into a 64*I transpose identity.
    Gate sections are interleaved with stream sections (x -> y_up ->
    s -> y_tar -> g -> geo) so the PE never idles long enough for the
    HAM clock gate to re-throttle it.
  * Row-sharded fp8 streams (scale 2^13): y_up = Up[ru]@x,
    y_tar = Tar[re]@s, geo = Geo[rp]@g, hg = Pu[rp]@y_up,
    trans = Src[rp]@y_tar.  y_up / y_tar transposed to natural fp8 and
    all-gathered (64KB in each) — the only two collectives; both hide
    behind the Geo/Pu streams.
  * Users are column-shard partials reduced on the HOST (free):
    usersT_partial = {hg,geo}[rp].T @ Up[:,rp].T via the own-block
    natural fp8 pois (built anyway for the output adds) against a
    streamed UpC — no third collective, no serial tail.
  * All DMAs ride the two HWDGE queues (sync = big streams, scalar =
    consts/cc/outputs); gpsimd only triggers collectives.  Queue
    order avoids head-of-line deadlocks (a trigger that gates a
    collective never sits behind a trigger that waits on consumers).

Scales: gate fp8 = 2^6*gate (via 64*I identity); matrices fp8 =
2^13*mat; y fp8 = 2^19*y; second-hop psum 2^32 -> f32 outputs are
UNSCALED (2^-32 folded into the output add); users psum = 2^19
(host descales).  The direct x-term is added from an f32 own-gate.
"""

import sys

if "/opt/trn_rl_repo" not in sys.path:
    sys.path.insert(0, "/opt/trn_rl_repo")

import numpy as np
import ml_dtypes

import concourse.bass as bass  # noqa: F401
import concourse.bacc as bacc
import concourse.mybir as mybir
import concourse.tile as tile
from concourse.bass_utils import run_bass_kernel_spmd

F32 = mybir.dt.float32
BF16 = mybir.dt.bfloat16
FP8 = mybir.dt.float8e4
SIG = mybir.ActivationFunctionType.Sigmoid
MULT = mybir.AluOpType.mult
ADD = mybir.AluOpType.add
BYPASS = mybir.AluOpType.bypass
DR = mybir.MatmulPerfMode.DoubleRow

NCORES = 8
P, U, E, D = 8192, 4096, 4096, 128
PP, UU, EE = P // NCORES, U // NCORES, E // NCORES  # 1024, 512, 512
KP, KU = P // 128, U // 128                         # 64, 32 k-tiles
NPP = KP // 2                                       # 32 k-pairs over P
NPU = KU // 2                                       # 16 k-pairs over U
RG = [list(range(NCORES))]

SX = 64.0         # gate fp8 scale 2^6 (folded into identS)
SM = 8192.0       # matrix fp8 scale 2^13
GEO_SCALE = 0.4   # ALPHA / 2 * T_GEO
S_HG_OUT = 2.0 ** -32               # psum 2^32 -> unscaled f32 output
S_GEO_OUT = GEO_SCALE * 2.0 ** -19  # psum 2^19 -> unscaled f32 output
S_USERS = 2.0 ** -19                # host descale for user partials

_CACHE: dict = {}


def _build_nc():
    nc = bacc.Bacc(
        "TRN2",
        target_bir_lowering=False,
        debug=False,
        enable_asserts=False,
        num_devices=NCORES,
    )

    # ---- per-core DRAM inputs -------------------------------------------
    wN = nc.dram_tensor("wN", [D, 3, D], BF16, kind="ExternalInput").ap()
    bT3 = nc.dram_tensor("bT3", [D, 3], F32, kind="ExternalInput").ap()
    ident = nc.dram_tensor("ident", [D, D], F32, kind="ExternalInput").ap()
    identSb = nc.dram_tensor("identSb", [D, D], BF16, kind="ExternalInput").ap()
    identSf = nc.dram_tensor("identSf", [D, D], F32, kind="ExternalInput").ap()
    peT = nc.dram_tensor("peT", [D, P], BF16, kind="ExternalInput").ap()
    peN = nc.dram_tensor("peN", [128, KP, D], BF16, kind="ExternalInput").ap()
    peTo_b = nc.dram_tensor("peTo_b", [D, PP], BF16, kind="ExternalInput").ap()
    # fp8 streams in paired k-tile layout [128, n_k/2, 2, n_out]
    UpT = nc.dram_tensor("UpT", [128, NPP, 2, UU], FP8, kind="ExternalInput").ap()
    TarT = nc.dram_tensor("TarT", [128, NPP, 2, EE], FP8, kind="ExternalInput").ap()
    GeoT = nc.dram_tensor("GeoT", [128, NPP, 2, PP], FP8, kind="ExternalInput").ap()
    PuT = nc.dram_tensor("PuT", [128, NPU, 2, PP], FP8, kind="ExternalInput").ap()
    SrcT = nc.dram_tensor("SrcT", [128, NPU, 2, PP], FP8, kind="ExternalInput").ap()
    # users stream: u-chunk-major [128, 8 u-chunks, 4 pairs, 2, 512]
    UpC = nc.dram_tensor(
        "UpC", [128, U // 512, PP // 256, 2, 512], FP8, kind="ExternalInput"
    ).ap()

    poisT_o = nc.dram_tensor("poisT_o", [3, D, PP], BF16, kind="ExternalOutput").ap()
    usersT_o = nc.dram_tensor("usersT_o", [D, 2, U], BF16, kind="ExternalOutput").ap()

    with tile.TileContext(nc) as tc:
        with (
            tc.tile_pool(name="const", bufs=1) as constp,
            tc.tile_pool(name="rhs", bufs=2) as rhsp,
            tc.tile_pool(name="stage", bufs=2) as stagep,
            tc.tile_pool(name="outp", bufs=2) as outp,
            tc.tile_pool(name="psacc", bufs=2, space="PSUM") as psacc,
            tc.tile_pool(name="pz", bufs=2, space="PSUM") as pzp,
            tc.tile_pool(name="dram", bufs=1, space="DRAM") as dramp,
        ):
            # ---- collective bounce buffers ------------------------------
            cc_w_in = dramp.tile([D, 3], F32, name="cc_w_in")
            cc_w_out = dramp.tile(
                [NCORES * D, 3], F32, addr_space="Shared", name="cc_w_out"
            )
            cc_yu_in = dramp.tile([128, 4, D], FP8, name="cc_yu_in")
            cc_yu_out = dramp.tile(
                [NCORES * 128, 4, D], FP8, addr_space="Shared", name="cc_yu_out"
            )
            cc_yt_in = dramp.tile([128, 4, D], FP8, name="cc_yt_in")
            cc_yt_out = dramp.tile(
                [NCORES * 128, 4, D], FP8, addr_space="Shared", name="cc_yt_out"
            )

            # ---- constants (scalar queue) + resident Up (sync queue) ----
            sb_w = constp.tile([D, 3, D], BF16, name="sb_w")
            nc.scalar.dma_start(sb_w[:], wN)
            sb_bT = constp.tile([D, 3], F32, name="sb_bT")
            nc.scalar.dma_start(sb_bT[:], bT3)
            sb_id = constp.tile([D, D], F32, name="sb_id")
            nc.scalar.dma_start(sb_id[:], ident)
            sb_idSb = constp.tile([D, D], BF16, name="sb_idSb")
            nc.scalar.dma_start(sb_idSb[:], identSb)
            sb_idSf = constp.tile([D, D], F32, name="sb_idSf")
            nc.scalar.dma_start(sb_idSf[:], identSf)
            sb_peTo_b = constp.tile([D, PP], BF16, name="sb_peTo_b")
            nc.scalar.dma_start(sb_peTo_b[:], peTo_b)
            sb_peT = constp.tile([D, P], BF16, name="sb_peT")
            nc.scalar.dma_start(sb_peT[:], peT)

            # warmup collective: absorbs the ~35us ncfw cold-start so the
            # real AllGather below runs at warm latency (~14us)
            nc.scalar.dma_start(cc_w_in[:], bT3)
            nc.gpsimd.collective_compute(
                "AllGather", BYPASS, replica_groups=RG,
                ins=[cc_w_in[:].opt()], outs=[cc_w_out[:].opt()],
            )

            sb_peN = constp.tile([128, KP, D], BF16, name="sb_peN")
            nc.sync.dma_start(sb_peN[:], peN)
            sb_up8 = constp.tile([128, NPP, 2, UU], FP8, name="sb_up8")
            nc.sync.dma_start(sb_up8[:], UpT)

            sb_gate8 = [
                constp.tile([128, KP, D], FP8, name=f"sb_gate8_{t}")
                for t in range(3)
            ]
            sb_gateT = [
                constp.tile([D, PP], F32, name=f"sb_gateT{t}") for t in range(3)
            ]
            sb_og8own = constp.tile([128, 2, 8, D], FP8, name="sb_og8own")

            def cast_eng(c):
                return nc.scalar if c % 2 == 0 else nc.vector

            def cast_copy(eng, dst, src):
                if eng is nc.scalar:
                    eng.copy(dst, src)
                else:
                    eng.tensor_copy(dst, src)

            def transpose_to(srcT, dst_slices, idn, dt=F32):
                """PE-transpose [D, n*128] into fp8 natural k-tiles.

                dst_slices: list of [128, 4, D] fp8 AP groups (4 tiles each).
                """
                for c, dst in enumerate(dst_slices):
                    pst = pzp.tile([128, 4, D], dt, tag="pz")
                    for m in range(4):
                        col = (4 * c + m) * 128
                        nc.tensor.transpose(
                            pst[:, m, :], srcT[:, col : col + 128], idn
                        )
                    cast_copy(cast_eng(c), dst, pst[:])

            def gate_full(t):
                """Full gate in natural fp8 k-tiles via bf16 panels.

                zT panel -> sigmoid (scalar) -> PE-transpose of sig with a
                64*I identity -> ONE fused vector mul peN * (64*sig.T)
                reading PSUM and writing fp8 (merges mul + cast, keeps
                scalar free for the sigmoids)."""
                for q in range(NCORES):
                    psg = psacc.tile([D, PP], F32, tag="acc")
                    for h in range(2):
                        cols = slice(1024 * q + 512 * h, 1024 * q + 512 * (h + 1))
                        nc.tensor.matmul(
                            psg[:, 512 * h : 512 * (h + 1)],
                            sb_w[:, t, :], sb_peT[:, cols],
                            start=True, stop=True,
                        )
                    sig = stagep.tile([D, PP], BF16, tag="sig")
                    nc.scalar.activation(
                        sig[:], psg[:], SIG, bias=sb_bT[:, t : t + 1]
                    )
                    for c in range(2):
                        pst = pzp.tile([128, 4, D], BF16, tag="pz")
                        for m in range(4):
                            col = (4 * c + m) * 128
                            nc.tensor.transpose(
                                pst[:, m, :], sig[:, col : col + 128], sb_idSb[:]
                            )
                        ks = slice(8 * q + 4 * c, 8 * q + 4 * c + 4)
                        nc.vector.tensor_mul(
                            sb_gate8[t][:, ks, :], sb_peN[:, ks, :], pst[:]
                        )

            def gate_own(t):
                """Own-block transposed gate, f32 (the direct x-term)."""
                psg = psacc.tile([D, PP], F32, tag="acc")
                for h in range(2):
                    cols = slice(512 * h, 512 * (h + 1))
                    nc.tensor.matmul(
                        psg[:, cols], sb_w[:, t, :], sb_peTo_b[:, cols],
                        start=True, stop=True,
                    )
                sigO = stagep.tile([D, PP], BF16, tag="sig")
                nc.scalar.activation(
                    sigO[:], psg[:], SIG, bias=sb_bT[:, t : t + 1]
                )
                nc.vector.tensor_mul(sb_gateT[t][:], sb_peTo_b[:], sigO[:])

            def gpair(t, p):
                return sb_gate8[t][:, 2 * p : 2 * p + 2, :]

            def stream_mm(lhs_fn, matT, n_pairs, ck, n_out, ps_slices, tag,
                          bufs=2):
                """Stream matT in ck-pair chunks, alternating the two HWDGE
                queues (sync even chunks, scalar odd) for DMA bandwidth."""
                for ci, c0 in enumerate(range(0, n_pairs, ck)):
                    chunk = rhsp.tile(
                        [128, ck, 2, n_out], FP8, tag=tag, name=tag, bufs=bufs
                    )
                    q = nc.sync if ci % 2 == 0 else nc.scalar
                    q.dma_start(chunk[:], matT[:, c0 : c0 + ck, :, :])
                    for kk in range(ck):
                        p = c0 + kk
                        for n, ps in enumerate(ps_slices):
                            nc.tensor.matmul(
                                ps, lhs_fn(p),
                                chunk[:, kk, :, 512 * n : 512 * (n + 1)],
                                start=(p == 0), stop=(p == n_pairs - 1),
                                perf_mode=DR,
                            )

            # ---- gate x, then y_up = HG_up[ru] @ x (Up resident) --------
            gate_full(0)
            gate_own(0)
            ps_yu = psacc.tile([D, UU], F32, tag="acc")
            for p in range(NPP):
                nc.tensor.matmul(
                    ps_yu[:], gpair(0, p), sb_up8[:, p, :, :],
                    start=(p == 0), stop=(p == NPP - 1), perf_mode=DR,
                )
            yuT = stagep.tile([D, UU], F32, tag="ysb", bufs=1)
            nc.scalar.copy(yuT[:], ps_yu[:])
            yu8o = stagep.tile([128, 4, D], FP8, tag="y8o")
            transpose_to(yuT[:], [yu8o[:]], sb_id[:])
            nc.scalar.dma_start(cc_yu_in[:], yu8o[:])
            nc.gpsimd.collective_compute(
                "AllGather", BYPASS, replica_groups=RG,
                ins=[cc_yu_in[:].opt()], outs=[cc_yu_out[:].opt()],
            )

            # ---- gate s, then y_tar = Tar[re] @ s (Tar streamed) --------
            gate_full(1)
            gate_own(1)
            ps_yt = psacc.tile([D, EE], F32, tag="acc")
            stream_mm(lambda p: gpair(1, p), TarT, NPP, 4, EE, [ps_yt[:]], "ck_tar")
            ytT = stagep.tile([D, EE], F32, tag="ysb", bufs=1)
            nc.scalar.copy(ytT[:], ps_yt[:])
            yt8o = stagep.tile([128, 4, D], FP8, tag="y8o")
            transpose_to(ytT[:], [yt8o[:]], sb_id[:])
            nc.sync.dma_start(cc_yt_in[:], yt8o[:])
            nc.gpsimd.collective_compute(
                "AllGather", BYPASS, replica_groups=RG,
                ins=[cc_yt_in[:].opt()], outs=[cc_yt_out[:].opt()],
            )

            # ---- gate g, then geo = g + 0.4 * Geo[rp] @ g ---------------
            gate_full(2)
            gate_own(2)
            ps_geo = psacc.tile([D, PP], F32, tag="acc")
            stream_mm(
                lambda p: gpair(2, p), GeoT, NPP, 4, PP,
                [ps_geo[:, 0:512], ps_geo[:, 512:1024]], "ck_geo",
            )
            geoT = outp.tile([D, PP], BF16, tag="out", name="geoT")
            nc.vector.scalar_tensor_tensor(
                geoT[:], ps_geo[:], S_GEO_OUT, sb_gateT[2][:], MULT, ADD
            )
            nc.scalar.dma_start(poisT_o[1], geoT[:])
            transpose_to(
                geoT[:],
                [sb_og8own[:, 1, 4 * c : 4 * c + 4, :] for c in range(2)],
                sb_idSb[:], dt=BF16,
            )

            # ---- hg = x + Pu[rp] @ y_up (Pu streamed) -------------------
            def ypair(sb_y8, p):
                j = 2 * (p % 2)
                return sb_y8[:, p // 2, j : j + 2, :]

            sb_yu8 = constp.tile([128, NCORES, 4, D], FP8, name="sb_yu8")
            nc.scalar.dma_start(
                sb_yu8[:],
                cc_yu_out[:].rearrange("(r p) j d -> p r j d", r=NCORES),
            )
            sb_yt8 = constp.tile([128, NCORES, 4, D], FP8, name="sb_yt8")
            nc.scalar.dma_start(
                sb_yt8[:],
                cc_yt_out[:].rearrange("(r p) j d -> p r j d", r=NCORES),
            )
            ps_hg = psacc.tile([D, PP], F32, tag="acc")
            stream_mm(
                lambda p: ypair(sb_yu8, p), PuT, NPU, 4, PP,
                [ps_hg[:, 0:512], ps_hg[:, 512:1024]], "ck_pu", bufs=3,
            )
            hgT = outp.tile([D, PP], BF16, tag="out", name="hgT")
            nc.vector.scalar_tensor_tensor(
                hgT[:], ps_hg[:], S_HG_OUT, sb_gateT[0][:], MULT, ADD
            )
            nc.scalar.dma_start(poisT_o[0], hgT[:])
            transpose_to(
                hgT[:],
                [sb_og8own[:, 0, 4 * c : 4 * c + 4, :] for c in range(2)],
                sb_idSb[:], dt=BF16,
            )

            # ---- trans = s + Src[rp] @ y_tar (Src streamed) -------------
            ps_tr = psacc.tile([D, PP], F32, tag="acc")
            stream_mm(
                lambda p: ypair(sb_yt8, p), SrcT, NPU, 2, PP,
                [ps_tr[:, 0:512], ps_tr[:, 512:1024]], "ck_src",
            )
            trT = outp.tile([D, PP], BF16, tag="out", name="trT")
            nc.vector.scalar_tensor_tensor(
                trT[:], ps_tr[:], S_HG_OUT, sb_gateT[1][:], MULT, ADD
            )
            nc.scalar.dma_start(poisT_o[2], trT[:])

            # ---- users partials: {hg,geo}[rp].T @ Up[:,rp].T ------------
            n_pairs_u = PP // 256
            for ucp in range(U // 1024):
                chunk = rhsp.tile([128, 8, 2, 512], FP8, tag="urhs",
                                  name="uchunk")
                q = nc.sync if ucp % 2 == 0 else nc.scalar
                q.dma_start(
                    chunk[:],
                    UpC[:, 2 * ucp : 2 * ucp + 2, :, :, :].rearrange(
                        "p u c two n -> p (u c) two n"
                    ),
                )
                for ui in range(2):
                    uc = 2 * ucp + ui
                    ps_u = [
                        psacc.tile([D, 512], F32, tag="uacc", name="ps_u")
                        for _ in range(2)
                    ]
                    for c0 in range(n_pairs_u):
                        for j in range(2):
                            nc.tensor.matmul(
                                ps_u[j][:],
                                sb_og8own[:, j, 2 * c0 : 2 * c0 + 2, :],
                                chunk[:, 4 * ui + c0, :, :],
                                start=(c0 == 0), stop=(c0 == n_pairs_u - 1),
                                perf_mode=DR,
                            )
                    users_uc = outp.tile([D, 2, 512], BF16, tag="uout",
                                         name="users_uc")
                    nc.vector.tensor_copy(users_uc[:, 0, :], ps_u[0][:])
                    nc.scalar.copy(users_uc[:, 1, :], ps_u[1][:])
                    nc.scalar.dma_start(
                        usersT_o[:, :, 512 * uc : 512 * (uc + 1)], users_uc[:]
                    )

    nc.compile()
    return nc


def _get_nc():
    if "nc" not in _CACHE:
        _CACHE["nc"] = _build_nc()
    return _CACHE["nc"]


def _pair_layout(matT, n_out):
    """[n_k*128, n_out] f32 -> fp8 paired k-tile layout [128, n_k/2, 2, n_out]."""
    n_k = matT.shape[0] // 128
    fp8 = ml_dtypes.float8_e4m3
    return np.ascontiguousarray(
        (matT * SM)
        .reshape(n_k // 2, 2, 128, n_out)
        .transpose(2, 0, 1, 3)
    ).astype(fp8)


def _shard_inputs(inputs):
    f32 = np.float32
    bf16 = ml_dtypes.bfloat16
    pe = np.asarray(inputs["poi_emb_weight"], f32)[:P]
    peT = np.ascontiguousarray(pe.T).astype(bf16)
    peN_nat = np.ascontiguousarray(
        pe.reshape(KP, 128, D).transpose(1, 0, 2)
    ).astype(bf16)
    wN = np.ascontiguousarray(
        np.stack(
            [
                np.asarray(inputs["w_gate_col"], f32),
                np.asarray(inputs["w_gate_seq"], f32),
                np.asarray(inputs["w_gate_geo"], f32),
            ]
        ).transpose(1, 0, 2)
    ).astype(bf16)
    b3 = np.stack(
        [
            np.asarray(inputs["b_gate_col"], f32)[0],
            np.asarray(inputs["b_gate_seq"], f32)[0],
            np.asarray(inputs["b_gate_geo"], f32)[0],
        ]
    )  # [3, D]
    bT3 = np.ascontiguousarray(b3.T)  # [D, 3] f32
    ident = np.eye(D, dtype=f32)
    identS = (SX * np.eye(D)).astype(f32)

    Up = np.asarray(inputs["HG_up"], f32)
    Pu = np.asarray(inputs["HG_pu"], f32)
    Tar = np.asarray(inputs["HG_poi_tar"], f32)
    Src = np.asarray(inputs["HG_poi_src"], f32)
    Geo = np.asarray(inputs["poi_geo_graph"], f32)

    in_maps = []
    for i in range(NCORES):
        rp = slice(PP * i, PP * (i + 1))
        ru = slice(UU * i, UU * (i + 1))
        re_ = slice(EE * i, EE * (i + 1))
        in_maps.append(
            {
                "wN": wN,
                "bT3": bT3,
                "ident": ident,
                "identSb": identS.astype(bf16),
                "identSf": identS,
                "peT": peT,
                "peN": peN_nat,
                "peTo_b": np.ascontiguousarray(peT[:, rp]),
                "UpT": _pair_layout(Up[ru].T, UU),
                "TarT": _pair_layout(Tar[re_].T, EE),
                "GeoT": _pair_layout(Geo[rp].T, PP),
                "PuT": _pair_layout(Pu[rp].T, PP),
                "SrcT": _pair_layout(Src[rp].T, PP),
                "UpC": np.ascontiguousarray(
                    (Up[:, rp].T * SM)
                    .reshape(PP // 256, 2, 128, U // 512, 512)
                    .transpose(2, 3, 0, 1, 4)
                ).astype(ml_dtypes.float8_e4m3),
            }
        )
    return in_maps


def _assemble(results, user_idx):
    f32 = np.float32
    hg = np.empty((P, D), f32)
    geo = np.empty((P, D), f32)
    tr = np.empty((P, D), f32)
    users_acc = np.zeros((D, 2, U), f32)
    for i in range(NCORES):
        rp = slice(PP * i, PP * (i + 1))
        pois = results[i]["poisT_o"].astype(f32)
        hg[rp] = pois[0].T
        geo[rp] = pois[1].T
        tr[rp] = pois[2].T
        users_acc += results[i]["usersT_o"].astype(f32)
    users_acc *= S_USERS
    hgu = users_acc[:, 0, :].T
    geou = users_acc[:, 1, :].T
    idx = np.asarray(user_idx)
    return np.concatenate([hg, geo, tr, hgu[idx], geou[idx]], axis=0)


def _run(inputs, trace=False, **spmd_kwargs):
    nc = _get_nc()
    in_maps = _shard_inputs(inputs)
    res = run_bass_kernel_spmd(
        nc, in_maps, list(range(NCORES)), trace=trace, **spmd_kwargs
    )
    return _assemble(res.results, inputs["user_idx"]), res


def kernel(**inputs):
    return _run(inputs)[0]


if __name__ == "__main__":
    import pickle

    with open("/tmp/inputs.pkl", "rb") as f:
        inputs = pickle.load(f)
    out = kernel(**inputs)
    exp = np.load("/tmp/expected.npy")
    rel = np.linalg.norm(out - exp) / np.linalg.norm(exp)
    print("Relative error:", rel)


# revision 52
# speedup vs baseline: 1.0117x; 1.0117x over previous
"""Trainium2 Bass kernel for nn_HODE_MDP (hypergraph ODE message passing).

Math (T_UP = T_GEO = T_P2P = 1.0, ALPHA = 0.8):
    pe  = poi_emb_weight[:-1]                      # [P, D]
    x/s/g = pe * sigmoid(pe @ W_t + b_t)           # col / seq / geo gates
    hg_pois    = x + HG_pu @ (HG_up @ x)
    geo_pois   = g + 0.4 * (poi_geo_graph @ g)
    trans_pois = s + HG_poi_src @ (HG_poi_tar @ s)
    hg_users   = (HG_up @ hg_pois)[user_idx]
    geo_users  = (HG_up @ geo_pois)[user_idx]
    out = concat([hg_pois, geo_pois, trans_pois, hg_users, geo_users])

Distribution (8 NeuronCores), v3 — two collectives only:
  * Full gates are computed on every core (collective latency on this
    part measured ~55us serial, far worse than replicating the work):
    bf16 panel pipeline over 1024-col panels of zT = W.T @ peT, sigmoid
    (scalar), pe*sig (vector), PE-transpose into natural fp8 k-tiles
    with the 2^6 fp8 scale folded into a 64*I transpose identity.
    Gate sections are interleaved with stream sections (x -> y_up ->
    s -> y_tar -> g -> geo) so the PE never idles long enough for the
    HAM clock gate to re-throttle it.
  * Row-sharded fp8 streams (scale 2^13): y_up = Up[ru]@x,
    y_tar = Tar[re]@s, geo = Geo[rp]@g, hg = Pu[rp]@y_up,
    trans = Src[rp]@y_tar.  y_up / y_tar transposed to natural fp8 and
    all-gathered (64KB in each) — the only two collectives; both hide
    behind the Geo/Pu streams.
  * Users are column-shard partials reduced on the HOST (free):
    usersT_partial = {hg,geo}[rp].T @ Up[:,rp].T via the own-block
    natural fp8 pois (built anyway for the output adds) against a
    streamed UpC — no third collective, no serial tail.
  * All DMAs ride the two HWDGE queues (sync = big streams, scalar =
    consts/cc/outputs); gpsimd only triggers collectives.  Queue
    order avoids head-of-line deadlocks (a trigger that gates a
    collective never sits behind a trigger that waits on consumers).

Scales: gate fp8 = 2^6*gate (via 64*I identity); matrices fp8 =
2^13*mat; y fp8 = 2^19*y; second-hop psum 2^32 -> f32 outputs are
UNSCALED (2^-32 folded into the output add); users psum = 2^19
(host descales).  The direct x-term is added from an f32 own-gate.
"""

import sys

if "/opt/trn_rl_repo" not in sys.path:
    sys.path.insert(0, "/opt/trn_rl_repo")

import numpy as np
import ml_dtypes

import concourse.bass as bass  # noqa: F401
import concourse.bacc as bacc
import concourse.mybir as mybir
import concourse.tile as tile
from concourse.bass_utils import run_bass_kernel_spmd

F32 = mybir.dt.float32
BF16 = mybir.dt.bfloat16
FP8 = mybir.dt.float8e4
SIG = mybir.ActivationFunctionType.Sigmoid
MULT = mybir.AluOpType.mult
ADD = mybir.AluOpType.add
BYPASS = mybir.AluOpType.bypass
DR = mybir.MatmulPerfMode.DoubleRow

NCORES = 8
P, U, E, D = 8192, 4096, 4096, 128
PP, UU, EE = P // NCORES, U // NCORES, E // NCORES  # 1024, 512, 512
KP, KU = P // 128, U // 128                         # 64, 32 k-tiles
NPP = KP // 2                                       # 32 k-pairs over P
NPU = KU // 2                                       # 16 k-pairs over U
RG = [list(range(NCORES))]

SX = 64.0         # gate fp8 scale 2^6 (folded into identS)
SM = 8192.0       # matrix fp8 scale 2^13
GEO_SCALE = 0.4   # ALPHA / 2 * T_GEO
S_HG_OUT = 2.0 ** -32               # psum 2^32 -> unscaled f32 output
S_GEO_OUT = GEO_SCALE * 2.0 ** -19  # psum 2^19 -> unscaled f32 output
S_USERS = 2.0 ** -19                # host descale for user partials

_CACHE: dict = {}


def _build_nc():
    nc = bacc.Bacc(
        "TRN2",
        target_bir_lowering=False,
        debug=False,
        enable_asserts=False,
        num_devices=NCORES,
    )

    # ---- per-core DRAM inputs -------------------------------------------
    wN = nc.dram_tensor("wN", [D, 3, D], BF16, kind="ExternalInput").ap()
    bT3 = nc.dram_tensor("bT3", [D, 3], F32, kind="ExternalInput").ap()
    ident = nc.dram_tensor("ident", [D, D], F32, kind="ExternalInput").ap()
    identSb = nc.dram_tensor("identSb", [D, D], BF16, kind="ExternalInput").ap()
    identSf = nc.dram_tensor("identSf", [D, D], F32, kind="ExternalInput").ap()
    peT = nc.dram_tensor("peT", [D, P], BF16, kind="ExternalInput").ap()
    peN = nc.dram_tensor("peN", [128, KP, D], BF16, kind="ExternalInput").ap()
    peTo_b = nc.dram_tensor("peTo_b", [D, PP], BF16, kind="ExternalInput").ap()
    # fp8 streams in paired k-tile layout [128, n_k/2, 2, n_out]
    UpT = nc.dram_tensor("UpT", [128, NPP, 2, UU], FP8, kind="ExternalInput").ap()
    TarT = nc.dram_tensor("TarT", [128, NPP, 2, EE], FP8, kind="ExternalInput").ap()
    GeoT = nc.dram_tensor("GeoT", [128, NPP, 2, PP], FP8, kind="ExternalInput").ap()
    PuT = nc.dram_tensor("PuT", [128, NPU, 2, PP], FP8, kind="ExternalInput").ap()
    SrcT = nc.dram_tensor("SrcT", [128, NPU, 2, PP], FP8, kind="ExternalInput").ap()
    # users stream: u-chunk-major [128, 8 u-chunks, 4 pairs, 2, 512]
    UpC = nc.dram_tensor(
        "UpC", [128, U // 512, PP // 256, 2, 512], FP8, kind="ExternalInput"
    ).ap()

    poisT_o = nc.dram_tensor("poisT_o", [3, D, PP], BF16, kind="ExternalOutput").ap()
    usersT_o = nc.dram_tensor("usersT_o", [D, 2, U], BF16, kind="ExternalOutput").ap()

    with tile.TileContext(nc) as tc:
        with (
            tc.tile_pool(name="const", bufs=1) as constp,
            tc.tile_pool(name="rhs", bufs=2) as rhsp,
            tc.tile_pool(name="stage", bufs=2) as stagep,
            tc.tile_pool(name="outp", bufs=2) as outp,
            tc.tile_pool(name="psacc", bufs=2, space="PSUM") as psacc,
            tc.tile_pool(name="pz", bufs=2, space="PSUM") as pzp,
            tc.tile_pool(name="dram", bufs=1, space="DRAM") as dramp,
        ):
            # ---- collective bounce buffers ------------------------------
            cc_w_in = dramp.tile([D, 3], F32, name="cc_w_in")
            cc_w_out = dramp.tile(
                [NCORES * D, 3], F32, addr_space="Shared", name="cc_w_out"
            )
            cc_y_in = dramp.tile([128, 2, 4, D], FP8, name="cc_y_in")
            cc_y_out = dramp.tile(
                [NCORES * 128, 2, 4, D], FP8, addr_space="Shared", name="cc_y_out"
            )

            # ---- constants (scalar queue) + resident Up (sync queue) ----
            sb_w = constp.tile([D, 3, D], BF16, name="sb_w")
            nc.scalar.dma_start(sb_w[:], wN)
            sb_bT = constp.tile([D, 3], F32, name="sb_bT")
            nc.scalar.dma_start(sb_bT[:], bT3)
            sb_id = constp.tile([D, D], F32, name="sb_id")
            nc.scalar.dma_start(sb_id[:], ident)
            sb_idSb = constp.tile([D, D], BF16, name="sb_idSb")
            nc.scalar.dma_start(sb_idSb[:], identSb)
            sb_idSf = constp.tile([D, D], F32, name="sb_idSf")
            nc.scalar.dma_start(sb_idSf[:], identSf)
            sb_peTo_b = constp.tile([D, PP], BF16, name="sb_peTo_b")
            nc.scalar.dma_start(sb_peTo_b[:], peTo_b)
            sb_peT = constp.tile([D, P], BF16, name="sb_peT")
            nc.scalar.dma_start(sb_peT[:], peT)

            # warmup collective: absorbs the ~35us ncfw cold-start so the
            # real AllGather below runs at warm latency (~14us)
            nc.scalar.dma_start(cc_w_in[:], bT3)
            nc.gpsimd.collective_compute(
                "AllGather", BYPASS, replica_groups=RG,
                ins=[cc_w_in[:].opt()], outs=[cc_w_out[:].opt()],
            )

            sb_peN = constp.tile([128, KP, D], BF16, name="sb_peN")
            nc.sync.dma_start(sb_peN[:], peN)
            sb_up8 = constp.tile([128, NPP, 2, UU], FP8, name="sb_up8")
            nc.sync.dma_start(sb_up8[:], UpT)

            sb_gate8 = [
                constp.tile([128, KP, D], FP8, name=f"sb_gate8_{t}")
                for t in range(3)
            ]
            sb_gateT = [
                constp.tile([D, PP], F32, name=f"sb_gateT{t}") for t in range(3)
            ]
            sb_og8own = constp.tile([128, 2, 8, D], FP8, name="sb_og8own")

            def cast_eng(c):
                return nc.scalar if c % 2 == 0 else nc.vector

            def cast_copy(eng, dst, src):
                if eng is nc.scalar:
                    eng.copy(dst, src)
                else:
                    eng.tensor_copy(dst, src)

            def transpose_to(srcT, dst_slices, idn, dt=F32):
                """PE-transpose [D, n*128] into fp8 natural k-tiles.

                dst_slices: list of [128, 4, D] fp8 AP groups (4 tiles each).
                """
                for c, dst in enumerate(dst_slices):
                    pst = pzp.tile([128, 4, D], dt, tag="pz")
                    for m in range(4):
                        col = (4 * c + m) * 128
                        nc.tensor.transpose(
                            pst[:, m, :], srcT[:, col : col + 128], idn
                        )
                    cast_copy(cast_eng(c), dst, pst[:])

            def gate_full(t):
                """Full gate in natural fp8 k-tiles via bf16 panels.

                zT panel -> sigmoid (scalar) -> PE-transpose of sig with a
                64*I identity -> ONE fused vector mul peN * (64*sig.T)
                reading PSUM and writing fp8 (merges mul + cast, keeps
                scalar free for the sigmoids)."""
                for q in range(NCORES):
                    psg = psacc.tile([D, PP], F32, tag="acc")
                    for h in range(2):
                        cols = slice(1024 * q + 512 * h, 1024 * q + 512 * (h + 1))
                        nc.tensor.matmul(
                            psg[:, 512 * h : 512 * (h + 1)],
                            sb_w[:, t, :], sb_peT[:, cols],
                            start=True, stop=True,
                        )
                    sig = stagep.tile([D, PP], BF16, tag="sig")
                    nc.scalar.activation(
                        sig[:], psg[:], SIG, bias=sb_bT[:, t : t + 1]
                    )
                    for c in range(2):
                        pst = pzp.tile([128, 4, D], BF16, tag="pz")
                        for m in range(4):
                            col = (4 * c + m) * 128
                            nc.tensor.transpose(
                                pst[:, m, :], sig[:, col : col + 128], sb_idSb[:]
                            )
                        ks = slice(8 * q + 4 * c, 8 * q + 4 * c + 4)
                        nc.vector.tensor_mul(
                            sb_gate8[t][:, ks, :], sb_peN[:, ks, :], pst[:]
                        )

            def gate_own(t):
                """Own-block transposed gate, f32 (the direct x-term)."""
                psg = psacc.tile([D, PP], F32, tag="acc")
                for h in range(2):
                    cols = slice(512 * h, 512 * (h + 1))
                    nc.tensor.matmul(
                        psg[:, cols], sb_w[:, t, :], sb_peTo_b[:, cols],
                        start=True, stop=True,
                    )
                sigO = stagep.tile([D, PP], BF16, tag="sig")
                nc.scalar.activation(
                    sigO[:], psg[:], SIG, bias=sb_bT[:, t : t + 1]
                )
                nc.vector.tensor_mul(sb_gateT[t][:], sb_peTo_b[:], sigO[:])

            def gpair(t, p):
                return sb_gate8[t][:, 2 * p : 2 * p + 2, :]

            def stream_mm(lhs_fn, matT, n_pairs, ck, n_out, ps_slices, tag,
                          bufs=2):
                """Stream matT in ck-pair chunks, alternating the two HWDGE
                queues (sync even chunks, scalar odd) for DMA bandwidth."""
                for ci, c0 in enumerate(range(0, n_pairs, ck)):
                    chunk = rhsp.tile(
                        [128, ck, 2, n_out], FP8, tag=tag, name=tag, bufs=bufs
                    )
                    q = nc.sync if ci % 2 == 0 else nc.scalar
                    q.dma_start(chunk[:], matT[:, c0 : c0 + ck, :, :])
                    for kk in range(ck):
                        p = c0 + kk
                        for n, ps in enumerate(ps_slices):
                            nc.tensor.matmul(
                                ps, lhs_fn(p),
                                chunk[:, kk, :, 512 * n : 512 * (n + 1)],
                                start=(p == 0), stop=(p == n_pairs - 1),
                                perf_mode=DR,
                            )

            # ---- gate x, then y_up = HG_up[ru] @ x (Up resident) --------
            gate_full(0)
            gate_own(0)
            ps_yu = psacc.tile([D, UU], F32, tag="acc")
            for p in range(NPP):
                nc.tensor.matmul(
                    ps_yu[:], gpair(0, p), sb_up8[:, p, :, :],
                    start=(p == 0), stop=(p == NPP - 1), perf_mode=DR,
                )
            yuT = stagep.tile([D, UU], F32, tag="ysb", bufs=1)
            nc.scalar.copy(yuT[:], ps_yu[:])
            yu8o = stagep.tile([128, 4, D], FP8, tag="y8o")
            transpose_to(yuT[:], [yu8o[:]], sb_id[:])
            nc.scalar.dma_start(cc_y_in[:, 0, :, :], yu8o[:])

            # ---- gate s, then y_tar = Tar[re] @ s (Tar streamed) --------
            gate_full(1)
            gate_own(1)
            ps_yt = psacc.tile([D, EE], F32, tag="acc")
            stream_mm(lambda p: gpair(1, p), TarT, NPP, 4, EE, [ps_yt[:]], "ck_tar")
            ytT = stagep.tile([D, EE], F32, tag="ysb", bufs=1)
            nc.scalar.copy(ytT[:], ps_yt[:])
            yt8o = stagep.tile([128, 4, D], FP8, tag="y8o")
            transpose_to(ytT[:], [yt8o[:]], sb_id[:])
            nc.scalar.dma_start(cc_y_in[:, 1, :, :], yt8o[:])

            # ---- single merged AllGather of y_up + y_tar ----------------
            nc.gpsimd.collective_compute(
                "AllGather", BYPASS, replica_groups=RG,
                ins=[cc_y_in[:].opt()], outs=[cc_y_out[:].opt()],
            )

            # ---- gate g, then geo = g + 0.4 * Geo[rp] @ g ---------------
            gate_full(2)
            gate_own(2)
            ps_geo = psacc.tile([D, PP], F32, tag="acc")
            stream_mm(
                lambda p: gpair(2, p), GeoT, NPP, 4, PP,
                [ps_geo[:, 0:512], ps_geo[:, 512:1024]], "ck_geo",
            )
            geoT = outp.tile([D, PP], BF16, tag="out", name="geoT")
            nc.vector.scalar_tensor_tensor(
                geoT[:], ps_geo[:], S_GEO_OUT, sb_gateT[2][:], MULT, ADD
            )
            nc.scalar.dma_start(poisT_o[1], geoT[:])
            transpose_to(
                geoT[:],
                [sb_og8own[:, 1, 4 * c : 4 * c + 4, :] for c in range(2)],
                sb_idSb[:], dt=BF16,
            )

            # ---- hg = x + Pu[rp] @ y_up (Pu streamed) -------------------
            def ypair(sb_y8, p):
                j = 2 * (p % 2)
                return sb_y8[:, p // 2, j : j + 2, :]

            sb_yu8 = constp.tile([128, NCORES, 4, D], FP8, name="sb_yu8")
            nc.scalar.dma_start(
                sb_yu8[:],
                cc_y_out[:].rearrange("(r p) i j d -> p i r j d", r=NCORES)[:, 0],
            )
            sb_yt8 = constp.tile([128, NCORES, 4, D], FP8, name="sb_yt8")
            nc.scalar.dma_start(
                sb_yt8[:],
                cc_y_out[:].rearrange("(r p) i j d -> p i r j d", r=NCORES)[:, 1],
            )
            ps_hg = psacc.tile([D, PP], F32, tag="acc")
            stream_mm(
                lambda p: ypair(sb_yu8, p), PuT, NPU, 4, PP,
                [ps_hg[:, 0:512], ps_hg[:, 512:1024]], "ck_pu", bufs=3,
            )
            hgT = outp.tile([D, PP], BF16, tag="out", name="hgT")
            nc.vector.scalar_tensor_tensor(
                hgT[:], ps_hg[:], S_HG_OUT, sb_gateT[0][:], MULT, ADD
            )
            nc.scalar.dma_start(poisT_o[0], hgT[:])
            transpose_to(
                hgT[:],
                [sb_og8own[:, 0, 4 * c : 4 * c + 4, :] for c in range(2)],
                sb_idSb[:], dt=BF16,
            )

            # ---- trans = s + Src[rp] @ y_tar (Src streamed) -------------
            ps_tr = psacc.tile([D, PP], F32, tag="acc")
            stream_mm(
                lambda p: ypair(sb_yt8, p), SrcT, NPU, 2, PP,
                [ps_tr[:, 0:512], ps_tr[:, 512:1024]], "ck_src",
            )
            trT = outp.tile([D, PP], BF16, tag="out", name="trT")
            nc.vector.scalar_tensor_tensor(
                trT[:], ps_tr[:], S_HG_OUT, sb_gateT[1][:], MULT, ADD
            )
            nc.scalar.dma_start(poisT_o[2], trT[:])

            # ---- users partials: {hg,geo}[rp].T @ Up[:,rp].T ------------
            n_pairs_u = PP // 256
            for ucp in range(U // 1024):
                chunk = rhsp.tile([128, 8, 2, 512], FP8, tag="urhs",
                                  name="uchunk")
                q = nc.sync if ucp % 2 == 0 else nc.scalar
                q.dma_start(
                    chunk[:],
                    UpC[:, 2 * ucp : 2 * ucp + 2, :, :, :].rearrange(
                        "p u c two n -> p (u c) two n"
                    ),
                )
                for ui in range(2):
                    uc = 2 * ucp + ui
                    ps_u = [
                        psacc.tile([D, 512], F32, tag="uacc", name="ps_u")
                        for _ in range(2)
                    ]
                    for c0 in range(n_pairs_u):
                        for j in range(2):
                            nc.tensor.matmul(
                                ps_u[j][:],
                                sb_og8own[:, j, 2 * c0 : 2 * c0 + 2, :],
                                chunk[:, 4 * ui + c0, :, :],
                                start=(c0 == 0), stop=(c0 == n_pairs_u - 1),
                                perf_mode=DR,
                            )
                    users_uc = outp.tile([D, 2, 512], BF16, tag="uout",
                                         name="users_uc")
                    nc.vector.tensor_copy(users_uc[:, 0, :], ps_u[0][:])
                    nc.scalar.copy(users_uc[:, 1, :], ps_u[1][:])
                    nc.scalar.dma_start(
                        usersT_o[:, :, 512 * uc : 512 * (uc + 1)], users_uc[:]
                    )

    nc.compile()
    return nc


def _get_nc():
    if "nc" not in _CACHE:
        _CACHE["nc"] = _build_nc()
    return _CACHE["nc"]


def _pair_layout(matT, n_out):
    """[n_k*128, n_out] f32 -> fp8 paired k-tile layout [128, n_k/2, 2, n_out]."""
    n_k = matT.shape[0] // 128
    fp8 = ml_dtypes.float8_e4m3
    return np.ascontiguousarray(
        (matT * SM)
        .reshape(n_k // 2, 2, 128, n_out)
        .transpose(2, 0, 1, 3)
    ).astype(fp8)


def _shard_inputs(inputs):
    f32 = np.float32
    bf16 = ml_dtypes.bfloat16
    pe = np.asarray(inputs["poi_emb_weight"], f32)[:P]
    peT = np.ascontiguousarray(pe.T).astype(bf16)
    peN_nat = np.ascontiguousarray(
        pe.reshape(KP, 128, D).transpose(1, 0, 2)
    ).astype(bf16)
    wN = np.ascontiguousarray(
        np.stack(
            [
                np.asarray(inputs["w_gate_col"], f32),
                np.asarray(inputs["w_gate_seq"], f32),
                np.asarray(inputs["w_gate_geo"], f32),
            ]
        ).transpose(1, 0, 2)
    ).astype(bf16)
    b3 = np.stack(
        [
            np.asarray(inputs["b_gate_col"], f32)[0],
            np.asarray(inputs["b_gate_seq"], f32)[0],
            np.asarray(inputs["b_gate_geo"], f32)[0],
        ]
    )  # [3, D]
    bT3 = np.ascontiguousarray(b3.T)  # [D, 3] f32
    ident = np.eye(D, dtype=f32)
    identS = (SX * np.eye(D)).astype(f32)

    Up = np.asarray(inputs["HG_up"], f32)
    Pu = np.asarray(inputs["HG_pu"], f32)
    Tar = np.asarray(inputs["HG_poi_tar"], f32)
    Src = np.asarray(inputs["HG_poi_src"], f32)
    Geo = np.asarray(inputs["poi_geo_graph"], f32)

    in_maps = []
    for i in range(NCORES):
        rp = slice(PP * i, PP * (i + 1))
        ru = slice(UU * i, UU * (i + 1))
        re_ = slice(EE * i, EE * (i + 1))
        in_maps.append(
            {
                "wN": wN,
                "bT3": bT3,
                "ident": ident,
                "identSb": identS.astype(bf16),
                "identSf": identS,
                "peT": peT,
                "peN": peN_nat,
                "peTo_b": np.ascontiguousarray(peT[:, rp]),
                "UpT": _pair_layout(Up[ru].T, UU),
                "TarT": _pair_layout(Tar[re_].T, EE),
                "GeoT": _pair_layout(Geo[rp].T, PP),
                "PuT": _pair_layout(Pu[rp].T, PP),
                "SrcT": _pair_layout(Src[rp].T, PP),
                "UpC": np.ascontiguousarray(
                    (Up[:, rp].T * SM)
                    .reshape(PP // 256, 2, 128, U // 512, 512)
                    .transpose(2, 3, 0, 1, 4)
                ).astype(ml_dtypes.float8_e4m3),
            }
        )
    return in_maps


def _assemble(results, user_idx):
    f32 = np.float32
    hg = np.empty((P, D), f32)
    geo = np.empty((P, D), f32)
    tr = np.empty((P, D), f32)
    users_acc = np.zeros((D, 2, U), f32)
    for i in range(NCORES):
        rp = slice(PP * i, PP * (i + 1))
        pois = results[i]["poisT_o"].astype(f32)
        hg[rp] = pois[0].T
        geo[rp] = pois[1].T
        tr[rp] = pois[2].T
        users_acc += results[i]["usersT_o"].astype(f32)
    users_acc *= S_USERS
    hgu = users_acc[:, 0, :].T
    geou = users_acc[:, 1, :].T
    idx = np.asarray(user_idx)
    return np.concatenate([hg, geo, tr, hgu[idx], geou[idx]], axis=0)


def _run(inputs, trace=False, **spmd_kwargs):
    nc = _get_nc()
    in_maps = _shard_inputs(inputs)
    res = run_bass_kernel_spmd(
        nc, in_maps, list(range(NCORES)), trace=trace, **spmd_kwargs
    )
    return _assemble(res.results, inputs["user_idx"]), res


def kernel(**inputs):
    return _run(inputs)[0]


if __name__ == "__main__":
    import pickle

    with open("/tmp/inputs.pkl", "rb") as f:
        inputs = pickle.load(f)
    out = kernel(**inputs)
    exp = np.load("/tmp/expected.npy")
    rel = np.linalg.norm(out - exp) / np.linalg.norm(exp)
    print("Relative error:", rel)
